# revision 18
# baseline (speedup 1.0000x reference)
"""Locally-connected (masked linear) layer for 8 TRN2 NeuronCores.

y = x @ (W * M)^T + b
  x: [4096, 4096] f32, W/M: [4096, 4096] f32, b: [4096] f32.

Strategy (tensor-parallel over out_features):
  - Each core owns a 512-row shard of W/M (and of the output columns).
  - Host premultiplies mw = W * M (exact masking), uploads x^T and mw^T
    contraction-major in bf16 so the device never transposes anything.
  - Device: PE matmuls accumulate y^T = mw^T.T @ x^T in fp32 PSUM,
    bias is added per-partition on evacuation, y^T shard DMAs out bf16.
  - DMA rings: x slabs stream on the sync (SP) HWDGE ring; weights, bias
    and outputs ride the scalar (Activation) ring so the x stream is
    never queued behind the 4MB weight upload (FIFO per ring).
  - The first pass interleaves batch groups 0+1 (all 8 PSUM banks) so
    the PE has 2x work per arriving weight tile while weights stream in;
    later groups run singly off the SBUF-resident weights.
  - Host concatenates the 8 y^T shards, upcasts and transposes back.
"""

import os

import numpy as np
import ml_dtypes

BATCH = 4096
IN_F = 4096
OUT_F = 4096
N_CORES = 8
O_SHARD = OUT_F // N_CORES  # 512
P = 128                     # SBUF partitions
BG = 512                    # batch columns per PSUM accumulation group
XCH = 4                     # k-tiles per x DMA slab

_BF16 = ml_dtypes.bfloat16
_NC = None
LAST_RESULT = None


def _ensure_axon_hooks_stub():
    """bass_utils' axon trace path imports antenv.axon_hooks, which this
    container's antenv stub lacks. Install a minimal registry so the
    import succeeds (hook None => bass_utils skips tracing gracefully)."""
    import sys
    import types

    try:
        import antenv.axon_hooks  # noqa: F401
        return
    except ImportError:
        pass
    import antenv

    mod = types.ModuleType("antenv.axon_hooks")
    mod._HOOK = None

    def set_axon_ntff_profile_hook(h):
        mod._HOOK = h

    def get_axon_ntff_profile_hook():
        return mod._HOOK

    mod.set_axon_ntff_profile_hook = set_axon_ntff_profile_hook
    mod.get_axon_ntff_profile_hook = get_axon_ntff_profile_hook
    antenv.axon_hooks = mod
    sys.modules["antenv.axon_hooks"] = mod


def _install_real_ntff_hook():
    """Wire the ctypes NTFF profiling hook (normally registered by the
    boot middleware) so run_bass_kernel_spmd(trace=True) works."""
    _ensure_axon_hooks_stub()
    import antenv.axon_hooks as ah

    if ah.get_axon_ntff_profile_hook() is None:
        try:
            from trn_agent_boot.trn_boot import _ntff_profile_via_ctypes

            hook = _ntff_profile_via_ctypes("/opt/axon/libaxon_pjrt.so")
            if hook is not None:
                ah.set_axon_ntff_profile_hook(hook)
        except Exception:
            pass
    try:
        import concourse.bass_utils as bu

        bu.upload_artifacts = lambda tmpdir: "local://" + str(tmpdir)
    except Exception:
        pass


def build_nc(batch=BATCH, in_f=IN_F, o_shard=O_SHARD, bg=BG, xch=XCH):
    import concourse.mybir as mybir
    from concourse import bacc
    from concourse.tile import TileContext

    p = P
    kt = in_f // p          # k tiles along contraction
    oc = o_shard // p       # out-feature chunks of 128
    ng = batch // bg        # batch groups
    bf16 = mybir.dt.bfloat16
    f32 = mybir.dt.float32

    nc = bacc.Bacc()
    xT = nc.declare_dram_parameter("xT", [in_f, batch], bf16, isOutput=False)
    # masked weights packed partition-major on the host:
    # mwP[p, k*o_shard + o] = (W*M)^T[k*128 + p, o] — so a span of
    # k-tiles is one DMA with multi-KB per-partition lines
    mwP = nc.declare_dram_parameter("mwP", [p, kt * o_shard], bf16,
                                    isOutput=False)
    bT = nc.declare_dram_parameter("bT", [p, oc], f32, isOutput=False)
    yT = nc.declare_dram_parameter("yT", [o_shard, batch], bf16,
                                   isOutput=True)

    xv = xT[:].rearrange("(c p) b -> p c b", p=p)   # [128, kt, batch]
    wv = mwP[:].rearrange("p (c o) -> p c o", c=kt)  # [128, kt, o_shard]

    # Batch-group schedule: the first two groups run as an interleaved
    # pair (2KB x DMA lines, 8 PSUM banks, 2x PE work per k-tile while
    # the weights stream in); the rest run singly — the 8-buffer PSUM
    # pool then rotates between disjoint bank sets, so a group's first
    # matmul never waits on the previous group's evacuation.
    assert ng >= 2 and ng % 2 == 0
    bg2 = 2 * bg

    with TileContext(nc) as tc:
        with tc.tile_pool(name="const", bufs=1) as cpool, \
             tc.tile_pool(name="xin", bufs=6) as xpool, \
             tc.tile_pool(name="acc", bufs=8, space="PSUM") as ppool, \
             tc.tile_pool(name="out", bufs=4) as opool:

            # masked weights, resident in SBUF for the whole kernel;
            # per-k-tile DMAs on the scalar ring pace with consumption.
            # k=0 rides the sync ring ahead of the x stream — the scalar
            # ring's DGE ramps later, and w[0] gates the first matmul.
            mw = cpool.tile([p, kt, o_shard], bf16)
            nc.sync.dma_start(out=mw[:, 0, :], in_=wv[:, 0, :])
            kbig = min(8, kt)
            for k in range(1, kbig):
                nc.scalar.dma_start(out=mw[:, k, :], in_=wv[:, k, :])
            if kbig < kt:
                # tail of the weights as one wide-line DMA (the packed
                # host layout makes it contiguous per partition)
                nc.scalar.dma_start(out=mw[:, kbig:kt, :],
                                    in_=wv[:, kbig:kt, :])

            bias_t = cpool.tile([p, oc], f32)
            nc.scalar.dma_start(out=bias_t, in_=bT[:])

            # boot tile: k-tile 0 of the first pair as its own small DMA
            # so the very first matmuls only wait on 256KB, not a full
            # slab (slab 0 still covers k=0 — the overlap is harmless)
            xboot = cpool.tile([p, 1, bg2], bf16)
            nc.sync.dma_start(out=xboot, in_=xv[:, 0:1, 0:bg2])

            # PE warmup: dummy matmuls on a zeroed tile while the first
            # DMAs are still in flight. The HAM clock gate needs ~3.4us
            # of sustained PE activity to lift the 1.2GHz cold throttle;
            # this burns that time during the DMA ramp so the real
            # matmul stream starts at the full 2.4GHz.
            garb = cpool.tile([p, bg], bf16)
            nc.vector.memset(garb, 0.0)
            ps_warm = ppool.tile([p, bg], f32, tag="ps", name="ps_warm")
            for _ in range(13):
                nc.tensor.matmul(ps_warm, garb[:, 0:p], garb,
                                 start=True, stop=True)

            def evac(j, psum, out_slice):
                # evacuations alternate DVE / ACT so two engines drain
                # PSUM banks in parallel (GpSimd cannot read PSUM)
                if j % 2 == 0:
                    nc.vector.tensor_scalar_add(
                        out=out_slice, in0=psum,
                        scalar1=bias_t[:, j:j + 1])
                else:
                    nc.scalar.add(out_slice, psum, bias_t[:, j:j + 1])

            # --- pair phase: groups 0 and 1 interleaved ---
            cols = slice(0, bg2)
            psums = {}
            for h in range(2):
                for j in range(oc):
                    psums[(h, j)] = ppool.tile(
                        [p, bg], f32, tag="ps", name=f"psp_{h}_{j}")
            # early slabs are half-size so delivery granularity matches
            # the cold-start DMA rate; later slabs amortize better
            slabs = []  # (start_k, n_k)
            k0 = 0
            while k0 < kt:
                ch = 2 if (k0 < 8 and kt > 8) else xch
                ch = min(ch, kt - k0)
                slabs.append((k0, ch))
                k0 += ch
            slab_of = {}
            for s in slabs:
                for k in range(s[0], s[0] + s[1]):
                    slab_of[k] = s
            xt = None
            for k in range(kt):
                s0, sn = slab_of[k]
                if k == s0:
                    xt = xpool.tile([p, sn, bg2], bf16, tag=f"xp{sn}",
                                    name=f"xp_{k}")
                    nc.sync.dma_start(out=xt, in_=xv[:, s0:s0 + sn, cols])
                for j in range(oc):
                    for h in range(2):
                        rhs_t = xboot if k == 0 else xt
                        kk = 0 if k == 0 else k - s0
                        nc.tensor.matmul(
                            psums[(h, j)],
                            mw[:, k, j * p:(j + 1) * p],
                            rhs_t[:, kk, h * bg:(h + 1) * bg],
                            start=(k == 0),
                            stop=(k == kt - 1),
                        )
            for j in range(oc):
                ot = opool.tile([p, bg2], bf16, tag="op", name=f"op_{j}")
                for h in range(2):
                    evac(j, psums[(h, j)], ot[:, h * bg:(h + 1) * bg])
                # out-DMA issues alternate rings so they don't serialize
                # on one engine behind the evacuations
                oeng = nc.sync if j % 2 == 0 else nc.scalar
                oeng.dma_start(out=yT[j * p:(j + 1) * p, cols], in_=ot)

            # --- single-group phase: groups 2..ng-1 ---
            for g in range(2, ng):
                gcols = slice(g * bg, (g + 1) * bg)
                psg = {}
                for j in range(oc):
                    psg[j] = ppool.tile(
                        [p, bg], f32, tag="ps", name=f"ps{g}_{j}")
                for k in range(kt):
                    if k % xch == 0:
                        xt = xpool.tile([p, xch, bg], bf16, tag="x",
                                        name=f"x{g}_{k}")
                        nc.sync.dma_start(
                            out=xt, in_=xv[:, k:k + xch, gcols])
                    for j in range(oc):
                        nc.tensor.matmul(
                            psg[j],
                            mw[:, k, j * p:(j + 1) * p],
                            xt[:, k % xch, :],
                            start=(k == 0),
                            stop=(k == kt - 1),
                        )
                for j in range(oc):
                    ot = opool.tile([p, bg], bf16, tag="o", name=f"o{g}_{j}")
                    evac(j, psg[j], ot)
                    oeng = nc.sync if j % 2 == 0 else nc.scalar
                    oeng.dma_start(
                        out=yT[j * p:(j + 1) * p, gcols], in_=ot)
    nc.finalize()
    return nc


def pack_weights(mw_shard_T):
    """[in_f, o_shard] -> partition-major [128, (in_f//128)*o_shard]."""
    in_f, o_shard = mw_shard_T.shape
    kt = in_f // P
    return np.ascontiguousarray(
        mw_shard_T.reshape(kt, P, o_shard).transpose(1, 0, 2)
        .reshape(P, kt * o_shard))


def _prep_in_maps(x, weight, bias, myFilter):
    oc = O_SHARD // P
    xTb = np.ascontiguousarray(np.asarray(x, np.float32).T).astype(_BF16)
    mw = np.asarray(weight, np.float32) * np.asarray(myFilter, np.float32)
    in_maps = []
    for c in range(N_CORES):
        rows = slice(c * O_SHARD, (c + 1) * O_SHARD)
        mwPb = pack_weights(mw[rows].T.astype(np.float32)).astype(_BF16)
        bTb = np.ascontiguousarray(
            np.asarray(bias, np.float32)[rows].reshape(oc, P).T)
        in_maps.append({"xT": xTb, "mwP": mwPb, "bT": bTb})
    return in_maps


def kernel(x, weight, bias, myFilter):
    global _NC, LAST_RESULT
    _ensure_axon_hooks_stub()
    from concourse.bass_utils import run_bass_kernel_spmd

    if _NC is None:
        _NC = build_nc()

    in_maps = _prep_in_maps(x, weight, bias, myFilter)

    kwargs = {}
    if os.environ.get("KERNEL_TRACE") == "1":
        _install_real_ntff_hook()
        kwargs["trace"] = True
        tdir = os.environ.get("KERNEL_TRACE_DIR")
        if tdir:
            kwargs["tmpdir"] = tdir

    res = run_bass_kernel_spmd(_NC, in_maps, list(range(N_CORES)), **kwargs)
    LAST_RESULT = res

    yT = np.concatenate(
        [np.asarray(res.results[c]["yT"]) for c in range(N_CORES)], axis=0)
    return np.ascontiguousarray(yT.T.astype(np.float32))


# revision 19
# speedup vs baseline: 1.0172x; 1.0172x over previous
"""Locally-connected (masked linear) layer for 8 TRN2 NeuronCores.

y = x @ (W * M)^T + b
  x: [4096, 4096] f32, W/M: [4096, 4096] f32, b: [4096] f32.

Strategy (tensor-parallel over out_features):
  - Each core owns a 512-row shard of W/M (and of the output columns).
  - Host premultiplies mw = W * M (exact masking), uploads x^T and mw^T
    contraction-major in bf16 so the device never transposes anything.
  - Device: PE matmuls accumulate y^T = mw^T.T @ x^T in fp32 PSUM,
    bias is added per-partition on evacuation, y^T shard DMAs out bf16.
  - DMA rings: x slabs stream on the sync (SP) HWDGE ring; weights, bias
    and outputs ride the scalar (Activation) ring so the x stream is
    never queued behind the 4MB weight upload (FIFO per ring).
  - The first pass interleaves batch groups 0+1 (all 8 PSUM banks) so
    the PE has 2x work per arriving weight tile while weights stream in;
    later groups run singly off the SBUF-resident weights.
  - Host concatenates the 8 y^T shards, upcasts and transposes back.
"""

import os

import numpy as np
import ml_dtypes

BATCH = 4096
IN_F = 4096
OUT_F = 4096
N_CORES = 8
O_SHARD = OUT_F // N_CORES  # 512
P = 128                     # SBUF partitions
BG = 512                    # batch columns per PSUM accumulation group
XCH = 4                     # k-tiles per x DMA slab

_BF16 = ml_dtypes.bfloat16
_NC = None
LAST_RESULT = None


def _ensure_axon_hooks_stub():
    """bass_utils' axon trace path imports antenv.axon_hooks, which this
    container's antenv stub lacks. Install a minimal registry so the
    import succeeds (hook None => bass_utils skips tracing gracefully)."""
    import sys
    import types

    try:
        import antenv.axon_hooks  # noqa: F401
        return
    except ImportError:
        pass
    import antenv

    mod = types.ModuleType("antenv.axon_hooks")
    mod._HOOK = None

    def set_axon_ntff_profile_hook(h):
        mod._HOOK = h

    def get_axon_ntff_profile_hook():
        return mod._HOOK

    mod.set_axon_ntff_profile_hook = set_axon_ntff_profile_hook
    mod.get_axon_ntff_profile_hook = get_axon_ntff_profile_hook
    antenv.axon_hooks = mod
    sys.modules["antenv.axon_hooks"] = mod


def _install_real_ntff_hook():
    """Wire the ctypes NTFF profiling hook (normally registered by the
    boot middleware) so run_bass_kernel_spmd(trace=True) works."""
    _ensure_axon_hooks_stub()
    import antenv.axon_hooks as ah

    if ah.get_axon_ntff_profile_hook() is None:
        try:
            from trn_agent_boot.trn_boot import _ntff_profile_via_ctypes

            hook = _ntff_profile_via_ctypes("/opt/axon/libaxon_pjrt.so")
            if hook is not None:
                ah.set_axon_ntff_profile_hook(hook)
        except Exception:
            pass
    try:
        import concourse.bass_utils as bu

        bu.upload_artifacts = lambda tmpdir: "local://" + str(tmpdir)
    except Exception:
        pass


def build_nc(batch=BATCH, in_f=IN_F, o_shard=O_SHARD, bg=BG, xch=XCH):
    import concourse.mybir as mybir
    from concourse import bacc
    from concourse.tile import TileContext

    p = P
    kt = in_f // p          # k tiles along contraction
    oc = o_shard // p       # out-feature chunks of 128
    ng = batch // bg        # batch groups
    bf16 = mybir.dt.bfloat16
    f32 = mybir.dt.float32

    nc = bacc.Bacc()
    xT = nc.declare_dram_parameter("xT", [in_f, batch], bf16, isOutput=False)
    # masked weights packed partition-major on the host:
    # mwP[p, k*o_shard + o] = (W*M)^T[k*128 + p, o] — so a span of
    # k-tiles is one DMA with multi-KB per-partition lines
    mwP = nc.declare_dram_parameter("mwP", [p, kt * o_shard], bf16,
                                    isOutput=False)
    bT = nc.declare_dram_parameter("bT", [p, oc], f32, isOutput=False)
    yT = nc.declare_dram_parameter("yT", [o_shard, batch], bf16,
                                   isOutput=True)

    xv = xT[:].rearrange("(c p) b -> p c b", p=p)   # [128, kt, batch]
    wv = mwP[:].rearrange("p (c o) -> p c o", c=kt)  # [128, kt, o_shard]

    # Batch-group schedule: the first two groups run as an interleaved
    # pair (2KB x DMA lines, 8 PSUM banks, 2x PE work per k-tile while
    # the weights stream in); the rest run singly — the 8-buffer PSUM
    # pool then rotates between disjoint bank sets, so a group's first
    # matmul never waits on the previous group's evacuation.
    assert ng >= 2 and ng % 2 == 0
    bg2 = 2 * bg

    with TileContext(nc) as tc:
        with tc.tile_pool(name="const", bufs=1) as cpool, \
             tc.tile_pool(name="xin", bufs=6) as xpool, \
             tc.tile_pool(name="acc", bufs=8, space="PSUM") as ppool, \
             tc.tile_pool(name="out", bufs=4) as opool:

            # masked weights, resident in SBUF for the whole kernel;
            # per-k-tile DMAs on the scalar ring pace with consumption.
            # k=0 rides the sync ring ahead of the x stream — the scalar
            # ring's DGE ramps later, and w[0] gates the first matmul.
            mw = cpool.tile([p, kt, o_shard], bf16)
            nc.sync.dma_start(out=mw[:, 0, :], in_=wv[:, 0, :])
            # per-k-tile chunks: wider-line bulk DMAs measurably starve
            # the x stream (SDMA round-robin is packet-granular)
            for k in range(1, kt):
                nc.scalar.dma_start(out=mw[:, k, :], in_=wv[:, k, :])

            bias_t = cpool.tile([p, oc], f32)
            nc.scalar.dma_start(out=bias_t, in_=bT[:])

            # boot tile: k-tile 0 of the first pair as its own small DMA
            # so the very first matmuls only wait on 256KB, not a full
            # slab (slab 0 still covers k=0 — the overlap is harmless)
            xboot = cpool.tile([p, 1, bg2], bf16)
            nc.sync.dma_start(out=xboot, in_=xv[:, 0:1, 0:bg2])

            # PE warmup: dummy matmuls on a zeroed tile while the first
            # DMAs are still in flight. The HAM clock gate needs ~3.4us
            # of sustained PE activity to lift the 1.2GHz cold throttle;
            # this burns that time during the DMA ramp so the real
            # matmul stream starts at the full 2.4GHz.
            garb = cpool.tile([p, bg], bf16)
            nc.vector.memset(garb, 0.0)
            ps_warm = ppool.tile([p, bg], f32, tag="ps", name="ps_warm")
            for _ in range(13):
                nc.tensor.matmul(ps_warm, garb[:, 0:p], garb,
                                 start=True, stop=True)

            def evac(j, psum, out_slice):
                # evacuations alternate DVE / ACT so two engines drain
                # PSUM banks in parallel (GpSimd cannot read PSUM)
                if j % 2 == 0:
                    nc.vector.tensor_scalar_add(
                        out=out_slice, in0=psum,
                        scalar1=bias_t[:, j:j + 1])
                else:
                    nc.scalar.add(out_slice, psum, bias_t[:, j:j + 1])

            # --- pair phase: groups 0 and 1 interleaved ---
            cols = slice(0, bg2)
            psums = {}
            for h in range(2):
                for j in range(oc):
                    psums[(h, j)] = ppool.tile(
                        [p, bg], f32, tag="ps", name=f"psp_{h}_{j}")
            # early slabs are half-size so delivery granularity matches
            # the cold-start DMA rate; later slabs amortize better
            slabs = []  # (start_k, n_k)
            k0 = 0
            while k0 < kt:
                ch = 2 if (k0 < 8 and kt > 8) else xch
                ch = min(ch, kt - k0)
                slabs.append((k0, ch))
                k0 += ch
            slab_of = {}
            for s in slabs:
                for k in range(s[0], s[0] + s[1]):
                    slab_of[k] = s
            xt = None
            for k in range(kt):
                s0, sn = slab_of[k]
                if k == s0:
                    xt = xpool.tile([p, sn, bg2], bf16, tag=f"xp{sn}",
                                    name=f"xp_{k}")
                    nc.sync.dma_start(out=xt, in_=xv[:, s0:s0 + sn, cols])
                for j in range(oc):
                    for h in range(2):
                        rhs_t = xboot if k == 0 else xt
                        kk = 0 if k == 0 else k - s0
                        nc.tensor.matmul(
                            psums[(h, j)],
                            mw[:, k, j * p:(j + 1) * p],
                            rhs_t[:, kk, h * bg:(h + 1) * bg],
                            start=(k == 0),
                            stop=(k == kt - 1),
                        )
            for j in range(oc):
                ot = opool.tile([p, bg2], bf16, tag="op", name=f"op_{j}")
                for h in range(2):
                    evac(j, psums[(h, j)], ot[:, h * bg:(h + 1) * bg])
                # out-DMA issues alternate rings so they don't serialize
                # on one engine behind the evacuations
                oeng = nc.sync if j % 2 == 0 else nc.scalar
                oeng.dma_start(out=yT[j * p:(j + 1) * p, cols], in_=ot)

            # --- single-group phase: groups 2..ng-1 ---
            for g in range(2, ng):
                gcols = slice(g * bg, (g + 1) * bg)
                psg = {}
                for j in range(oc):
                    psg[j] = ppool.tile(
                        [p, bg], f32, tag="ps", name=f"ps{g}_{j}")
                for k in range(kt):
                    if k % xch == 0:
                        xt = xpool.tile([p, xch, bg], bf16, tag="x",
                                        name=f"x{g}_{k}")
                        nc.sync.dma_start(
                            out=xt, in_=xv[:, k:k + xch, gcols])
                    for j in range(oc):
                        nc.tensor.matmul(
                            psg[j],
                            mw[:, k, j * p:(j + 1) * p],
                            xt[:, k % xch, :],
                            start=(k == 0),
                            stop=(k == kt - 1),
                        )
                for j in range(oc):
                    ot = opool.tile([p, bg], bf16, tag="o", name=f"o{g}_{j}")
                    evac(j, psg[j], ot)
                    oeng = nc.sync if j % 2 == 0 else nc.scalar
                    oeng.dma_start(
                        out=yT[j * p:(j + 1) * p, gcols], in_=ot)
    nc.finalize()
    return nc


def pack_weights(mw_shard_T):
    """[in_f, o_shard] -> partition-major [128, (in_f//128)*o_shard]."""
    in_f, o_shard = mw_shard_T.shape
    kt = in_f // P
    return np.ascontiguousarray(
        mw_shard_T.reshape(kt, P, o_shard).transpose(1, 0, 2)
        .reshape(P, kt * o_shard))


def _prep_in_maps(x, weight, bias, myFilter):
    oc = O_SHARD // P
    xTb = np.ascontiguousarray(np.asarray(x, np.float32).T).astype(_BF16)
    mw = np.asarray(weight, np.float32) * np.asarray(myFilter, np.float32)
    in_maps = []
    for c in range(N_CORES):
        rows = slice(c * O_SHARD, (c + 1) * O_SHARD)
        mwPb = pack_weights(mw[rows].T.astype(np.float32)).astype(_BF16)
        bTb = np.ascontiguousarray(
            np.asarray(bias, np.float32)[rows].reshape(oc, P).T)
        in_maps.append({"xT": xTb, "mwP": mwPb, "bT": bTb})
    return in_maps


def kernel(x, weight, bias, myFilter):
    global _NC, LAST_RESULT
    _ensure_axon_hooks_stub()
    from concourse.bass_utils import run_bass_kernel_spmd

    if _NC is None:
        _NC = build_nc()

    in_maps = _prep_in_maps(x, weight, bias, myFilter)

    kwargs = {}
    if os.environ.get("KERNEL_TRACE") == "1":
        _install_real_ntff_hook()
        kwargs["trace"] = True
        tdir = os.environ.get("KERNEL_TRACE_DIR")
        if tdir:
            kwargs["tmpdir"] = tdir

    res = run_bass_kernel_spmd(_NC, in_maps, list(range(N_CORES)), **kwargs)
    LAST_RESULT = res

    yT = np.concatenate(
        [np.asarray(res.results[c]["yT"]) for c in range(N_CORES)], axis=0)
    return np.ascontiguousarray(yT.T.astype(np.float32))


# revision 23
# speedup vs baseline: 1.0379x; 1.0204x over previous
"""Locally-connected (masked linear) layer for 8 TRN2 NeuronCores.

y = x @ (W * M)^T + b
  x: [4096, 4096] f32, W/M: [4096, 4096] f32, b: [4096] f32.

Strategy (tensor-parallel over out_features):
  - Each core owns a 512-row shard of W/M (and of the output columns).
  - Host premultiplies mw = W * M (exact masking), uploads x^T and mw^T
    contraction-major in bf16 so the device never transposes anything.
  - Device: PE matmuls accumulate y^T = mw^T.T @ x^T in fp32 PSUM,
    bias is added per-partition on evacuation, y^T shard DMAs out bf16.
  - DMA rings: x slabs stream on the sync (SP) HWDGE ring; weights, bias
    and outputs ride the scalar (Activation) ring so the x stream is
    never queued behind the 4MB weight upload (FIFO per ring).
  - The first pass interleaves batch groups 0+1 (all 8 PSUM banks) so
    the PE has 2x work per arriving weight tile while weights stream in;
    later groups run singly off the SBUF-resident weights.
  - Host concatenates the 8 y^T shards, upcasts and transposes back.
"""

import os

import numpy as np
import ml_dtypes

BATCH = 4096
IN_F = 4096
OUT_F = 4096
N_CORES = 8
O_SHARD = OUT_F // N_CORES  # 512
P = 128                     # SBUF partitions
BG = 512                    # batch columns per PSUM accumulation group
XCH = 4                     # k-tiles per x DMA slab

_BF16 = ml_dtypes.bfloat16
_NC = None
LAST_RESULT = None


def _ensure_axon_hooks_stub():
    """bass_utils' axon trace path imports antenv.axon_hooks, which this
    container's antenv stub lacks. Install a minimal registry so the
    import succeeds (hook None => bass_utils skips tracing gracefully)."""
    import sys
    import types

    try:
        import antenv.axon_hooks  # noqa: F401
        return
    except ImportError:
        pass
    import antenv

    mod = types.ModuleType("antenv.axon_hooks")
    mod._HOOK = None

    def set_axon_ntff_profile_hook(h):
        mod._HOOK = h

    def get_axon_ntff_profile_hook():
        return mod._HOOK

    mod.set_axon_ntff_profile_hook = set_axon_ntff_profile_hook
    mod.get_axon_ntff_profile_hook = get_axon_ntff_profile_hook
    antenv.axon_hooks = mod
    sys.modules["antenv.axon_hooks"] = mod


def _install_real_ntff_hook():
    """Wire the ctypes NTFF profiling hook (normally registered by the
    boot middleware) so run_bass_kernel_spmd(trace=True) works."""
    _ensure_axon_hooks_stub()
    import antenv.axon_hooks as ah

    if ah.get_axon_ntff_profile_hook() is None:
        try:
            from trn_agent_boot.trn_boot import _ntff_profile_via_ctypes

            hook = _ntff_profile_via_ctypes("/opt/axon/libaxon_pjrt.so")
            if hook is not None:
                ah.set_axon_ntff_profile_hook(hook)
        except Exception:
            pass
    try:
        import concourse.bass_utils as bu

        bu.upload_artifacts = lambda tmpdir: "local://" + str(tmpdir)
    except Exception:
        pass


def build_nc(batch=BATCH, in_f=IN_F, o_shard=O_SHARD, bg=BG, xch=XCH):
    import concourse.mybir as mybir
    from concourse import bacc
    from concourse.tile import TileContext

    p = P
    kt = in_f // p          # k tiles along contraction
    oc = o_shard // p       # out-feature chunks of 128
    ng = batch // bg        # batch groups
    bf16 = mybir.dt.bfloat16
    f32 = mybir.dt.float32

    nc = bacc.Bacc()
    xT = nc.declare_dram_parameter("xT", [in_f, batch], bf16, isOutput=False)
    # masked weights packed partition-major on the host:
    # mwP[p, k*o_shard + o] = (W*M)^T[k*128 + p, o] — so a span of
    # k-tiles is one DMA with multi-KB per-partition lines
    mwP = nc.declare_dram_parameter("mwP", [p, kt * o_shard], bf16,
                                    isOutput=False)
    bT = nc.declare_dram_parameter("bT", [p, oc], f32, isOutput=False)
    yT = nc.declare_dram_parameter("yT", [o_shard, batch], bf16,
                                   isOutput=True)

    xv = xT[:].rearrange("(c p) b -> p c b", p=p)   # [128, kt, batch]
    wv = mwP[:].rearrange("p (c o) -> p c o", c=kt)  # [128, kt, o_shard]

    # Batch-group schedule: the first two groups run as an interleaved
    # pair (2KB x DMA lines, 8 PSUM banks, 2x PE work per k-tile while
    # the weights stream in); the rest run singly — the 8-buffer PSUM
    # pool then rotates between disjoint bank sets, so a group's first
    # matmul never waits on the previous group's evacuation.
    assert ng >= 2 and ng % 2 == 0
    bg2 = 2 * bg

    with TileContext(nc) as tc:
        with tc.tile_pool(name="const", bufs=1) as cpool, \
             tc.tile_pool(name="xin", bufs=6) as xpool, \
             tc.tile_pool(name="acc", bufs=8, space="PSUM") as ppool, \
             tc.tile_pool(name="out", bufs=4) as opool:

            # masked weights, resident in SBUF for the whole kernel;
            # per-k-tile DMAs on the scalar ring pace with consumption.
            # k=0 rides the sync ring ahead of the x stream — the scalar
            # ring's DGE ramps later, and w[0] gates the first matmul.
            mw = cpool.tile([p, kt, o_shard], bf16)
            nc.sync.dma_start(out=mw[:, 0, :], in_=wv[:, 0, :])
            # per-k-tile chunks: wider-line bulk DMAs measurably starve
            # the x stream (SDMA round-robin is packet-granular). The
            # back half of the weights is deferred until after the pair
            # phase's x slabs on the sync ring — it isn't consumed until
            # t ~ 40us, and deferring it halves the weight bandwidth
            # demand in the contended DMA-ramp window.
            kdefer = kt // 2 if kt > 16 else kt
            for k in range(1, kdefer):
                nc.scalar.dma_start(out=mw[:, k, :], in_=wv[:, k, :])

            bias_t = cpool.tile([p, oc], f32)
            nc.scalar.dma_start(out=bias_t, in_=bT[:])

            # boot tile: k-tile 0 of the first pair as its own small DMA
            # so the very first matmuls only wait on 256KB, not a full
            # slab (slab 0 still covers k=0 — the overlap is harmless)
            xboot = cpool.tile([p, 1, bg2], bf16)
            nc.sync.dma_start(out=xboot, in_=xv[:, 0:1, 0:bg2])

            # PE warmup: dummy matmuls on a zeroed tile while the first
            # DMAs are still in flight. The HAM clock gate needs ~3.4us
            # of sustained PE activity to lift the 1.2GHz cold throttle;
            # this burns that time during the DMA ramp so the real
            # matmul stream starts at the full 2.4GHz.
            garb = cpool.tile([p, bg], bf16)
            nc.vector.memset(garb, 0.0)
            ps_warm = ppool.tile([p, bg], f32, tag="ps", name="ps_warm")
            for _ in range(13):
                nc.tensor.matmul(ps_warm, garb[:, 0:p], garb,
                                 start=True, stop=True)

            def evac(j, psum, out_slice):
                # evacuations alternate DVE / ACT so two engines drain
                # PSUM banks in parallel (GpSimd cannot read PSUM)
                if j % 2 == 0:
                    nc.vector.tensor_scalar_add(
                        out=out_slice, in0=psum,
                        scalar1=bias_t[:, j:j + 1])
                else:
                    nc.scalar.add(out_slice, psum, bias_t[:, j:j + 1])

            # --- pair phase: groups 0 and 1 interleaved ---
            cols = slice(0, bg2)
            psums = {}
            for h in range(2):
                for j in range(oc):
                    psums[(h, j)] = ppool.tile(
                        [p, bg], f32, tag="ps", name=f"psp_{h}_{j}")
            # early slabs are half-size so delivery granularity matches
            # the cold-start DMA rate; later slabs amortize better
            slabs = []  # (start_k, n_k)
            k0 = 0
            while k0 < kt:
                ch = 2 if (k0 < 8 and kt > 8) else xch
                ch = min(ch, kt - k0)
                slabs.append((k0, ch))
                k0 += ch
            slab_of = {}
            for s in slabs:
                for k in range(s[0], s[0] + s[1]):
                    slab_of[k] = s
            xt = None
            for k in range(kt):
                s0, sn = slab_of[k]
                if k == s0:
                    xt = xpool.tile([p, sn, bg2], bf16, tag=f"xp{sn}",
                                    name=f"xp_{k}")
                    nc.sync.dma_start(out=xt, in_=xv[:, s0:s0 + sn, cols])
                    # deferred back-half weight chunks ride the sync
                    # ring just-in-time, one slab ahead of consumption —
                    # this keeps them out of the contended early window
                    for kw in range(max(s0 + sn, kdefer),
                                    min(s0 + 2 * sn, kt)):
                        nc.sync.dma_start(out=mw[:, kw, :],
                                          in_=wv[:, kw, :])
                for j in range(oc):
                    for h in range(2):
                        rhs_t = xboot if k == 0 else xt
                        kk = 0 if k == 0 else k - s0
                        nc.tensor.matmul(
                            psums[(h, j)],
                            mw[:, k, j * p:(j + 1) * p],
                            rhs_t[:, kk, h * bg:(h + 1) * bg],
                            start=(k == 0),
                            stop=(k == kt - 1),
                        )
            for j in range(oc):
                ot = opool.tile([p, bg2], bf16, tag="op", name=f"op_{j}")
                for h in range(2):
                    evac(j, psums[(h, j)], ot[:, h * bg:(h + 1) * bg])
                # out-DMA issues alternate rings so they don't serialize
                # on one engine behind the evacuations
                oeng = nc.sync if j % 2 == 0 else nc.scalar
                oeng.dma_start(out=yT[j * p:(j + 1) * p, cols], in_=ot)

            # --- single-group phase: groups 2..ng-1 ---
            for g in range(2, ng):
                gcols = slice(g * bg, (g + 1) * bg)
                psg = {}
                for j in range(oc):
                    psg[j] = ppool.tile(
                        [p, bg], f32, tag="ps", name=f"ps{g}_{j}")
                for k in range(kt):
                    if k % xch == 0:
                        xt = xpool.tile([p, xch, bg], bf16, tag="x",
                                        name=f"x{g}_{k}")
                        nc.sync.dma_start(
                            out=xt, in_=xv[:, k:k + xch, gcols])
                    for j in range(oc):
                        nc.tensor.matmul(
                            psg[j],
                            mw[:, k, j * p:(j + 1) * p],
                            xt[:, k % xch, :],
                            start=(k == 0),
                            stop=(k == kt - 1),
                        )
                for j in range(oc):
                    ot = opool.tile([p, bg], bf16, tag="o", name=f"o{g}_{j}")
                    evac(j, psg[j], ot)
                    oeng = nc.sync if j % 2 == 0 else nc.scalar
                    oeng.dma_start(
                        out=yT[j * p:(j + 1) * p, gcols], in_=ot)
    nc.finalize()
    return nc


def pack_weights(mw_shard_T):
    """[in_f, o_shard] -> partition-major [128, (in_f//128)*o_shard]."""
    in_f, o_shard = mw_shard_T.shape
    kt = in_f // P
    return np.ascontiguousarray(
        mw_shard_T.reshape(kt, P, o_shard).transpose(1, 0, 2)
        .reshape(P, kt * o_shard))


def _prep_in_maps(x, weight, bias, myFilter):
    oc = O_SHARD // P
    xTb = np.ascontiguousarray(np.asarray(x, np.float32).T).astype(_BF16)
    mw = np.asarray(weight, np.float32) * np.asarray(myFilter, np.float32)
    in_maps = []
    for c in range(N_CORES):
        rows = slice(c * O_SHARD, (c + 1) * O_SHARD)
        mwPb = pack_weights(mw[rows].T.astype(np.float32)).astype(_BF16)
        bTb = np.ascontiguousarray(
            np.asarray(bias, np.float32)[rows].reshape(oc, P).T)
        in_maps.append({"xT": xTb, "mwP": mwPb, "bT": bTb})
    return in_maps


def kernel(x, weight, bias, myFilter):
    global _NC, LAST_RESULT
    _ensure_axon_hooks_stub()
    from concourse.bass_utils import run_bass_kernel_spmd

    if _NC is None:
        _NC = build_nc()

    in_maps = _prep_in_maps(x, weight, bias, myFilter)

    kwargs = {}
    if os.environ.get("KERNEL_TRACE") == "1":
        _install_real_ntff_hook()
        kwargs["trace"] = True
        tdir = os.environ.get("KERNEL_TRACE_DIR")
        if tdir:
            kwargs["tmpdir"] = tdir

    res = run_bass_kernel_spmd(_NC, in_maps, list(range(N_CORES)), **kwargs)
    LAST_RESULT = res

    yT = np.concatenate(
        [np.asarray(res.results[c]["yT"]) for c in range(N_CORES)], axis=0)
    return np.ascontiguousarray(yT.T.astype(np.float32))
